# revision 1
# baseline (speedup 1.0000x reference)
"""Trainium2 Bass kernel: batched 2D DCT-II (unnormalized), x: (8, 2048, 2048) f32.

Math: per image X, the unnormalized 2D DCT-II is Z = C @ X @ C^T with
C[k,n] = cos(pi*(2n+1)*k/(2N)).  Let D = C^T.  Using the PE's
out = lhsT.T @ rhs semantics with the *data* as the stationary operand and D as
the moving operand, the two passes chain with no transposes:
    pass 1:  T = X^T @ D      (lhsT = X tiles,  rhs = D)   -> T[c, f]
    pass 2:  Z = T^T @ D      (lhsT = T tiles,  rhs = D)   -> Z = D^T X D = C X C^T

Sharding: batch dim 8 -> one image per NeuronCore (data parallel, no comms).

Dtype modes:
  "f32r"  - single fp32r matmul per term (full PE rate; TF32-like 11-bit
            mantissa operand rounding; ~2e-4 relative-to-absmax error).
  "split" - hi/lo bf16 decomposition, 3 matmuls per term (~5e-6 error, 3x cost).
"""

import numpy as np
from contextlib import ExitStack

import concourse.bass as bass
import concourse.bacc as bacc
import concourse.tile as tile
from concourse import mybir
from concourse.bass_utils import run_bass_kernel_spmd

F32 = mybir.dt.float32
F32R = mybir.dt.float32r
BF16 = mybir.dt.bfloat16

import os

MODE = os.environ.get("DCT_MODE", "bfly")  # "bfly", "f32r", or "split"

B = 8          # batch == n_cores
N = 2048       # image is N x N
P = 128        # partitions
KT = N // P    # 16 k-tiles along any contraction
FC = 512       # chunk width (pass-1 f-chunk, pass-2 g-chunk, PSUM bank)
NFC = N // FC  # 4 chunks
H = N // 2     # butterfly half size
KT2 = H // P   # 8 k-tiles at half contraction


def _round_f32r(a: np.ndarray) -> np.ndarray:
    """fp32r = round-to-nearest, 11 explicit mantissa bits (drop low 12)."""
    b = np.ascontiguousarray(a, dtype=np.float32).view(np.uint32)
    r = ((b + np.uint32(0x800)) & np.uint32(0xFFFFF000)).view(np.float32)
    return r


def _split_bf16(a: np.ndarray):
    import ml_dtypes

    hi = a.astype(ml_dtypes.bfloat16)
    lo = (a - hi.astype(np.float32)).astype(ml_dtypes.bfloat16)
    return hi, lo


def _dct_matrix_d() -> np.ndarray:
    # D[n, k] = cos(pi * (2n+1) * k / (2N)), exact in float64
    n = np.arange(N, dtype=np.float64)[:, None]
    k = np.arange(N, dtype=np.float64)[None, :]
    d = np.cos(np.pi * (2.0 * n + 1.0) * k / (2.0 * N))
    return d.astype(np.float32)


def _build_f32r() -> bass.Bass:
    """fp32r two-pass DCT with the intermediate T round-tripped via DRAM.

    Pass 1 streams X once (one column-block per chain, all 4 f-chunks while
    the block is resident).  T chunks are written back to a DRAM scratch and
    re-streamed as pass-2 stationary tiles.  D stays resident in SBUF.
    """
    nc = bacc.Bacc(None, target_bir_lowering=False)
    x_ext = nc.declare_dram_parameter("x", [N, N], F32R, isOutput=False)
    d_ext = nc.declare_dram_parameter("d", [N, N], F32R, isOutput=False)
    z_ext = nc.declare_dram_parameter("z", [N, N], F32, isOutput=True)

    with ExitStack() as ctx:
        tc = ctx.enter_context(tile.TileContext(nc))
        d_pool = ctx.enter_context(tc.tile_pool(name="d", bufs=1))
        x_pool = ctx.enter_context(tc.tile_pool(name="x", bufs=3))
        t_pool = ctx.enter_context(tc.tile_pool(name="t", bufs=6))
        z_pool = ctx.enter_context(tc.tile_pool(name="z", bufs=3))
        dram = ctx.enter_context(tc.tile_pool(name="dram", bufs=1, space="DRAM"))
        ps1 = ctx.enter_context(tc.tile_pool(name="ps1", bufs=4, space="PSUM"))
        ps2 = ctx.enter_context(tc.tile_pool(name="ps2", bufs=4, space="PSUM"))

        t_dram = dram.tile([N, N], F32R, name="t_dram")

        # First column-block of X loads before D so pass 1 starts early.
        d_sb = [
            d_pool.tile([P, N], F32R, tag=f"d{t}", name=f"d{t}") for t in range(KT)
        ]

        def load_x(cb):
            xt = x_pool.tile([P, N], F32R, tag="x", name="xt")
            nc.sync.dma_start(
                xt[:].rearrange("p (t m) -> p t m", t=KT),
                x_ext[:, cb * P : (cb + 1) * P].rearrange("(t p) m -> p t m", p=P),
            )
            return xt

        x0 = load_x(0)
        # D f-chunk 0 for all 16 row-tiles (pass-1 chain 0 needs only these)
        for fcol in range(NFC):
            for t in range(KT):
                nc.sync.dma_start(
                    d_sb[t][:, fcol * FC : (fcol + 1) * FC],
                    d_ext[t * P : (t + 1) * P, fcol * FC : (fcol + 1) * FC],
                )
            if fcol == 0:
                # remaining D chunks stream behind pass-1 compute
                pass

        # ---- pass 1: per column-block cb, all f-chunks: T[cb,:] = (X^T D)[cb,:]
        for cb in range(KT):
            xt = x0 if cb == 0 else load_x(cb)
            for fc in range(NFC):
                pt = ps1.tile([P, FC], F32, tag="ps1", name="pt")
                for rt in range(KT):
                    nc.tensor.matmul(
                        pt[:],
                        lhsT=xt[:, rt * P : (rt + 1) * P],
                        rhs=d_sb[rt][:, fc * FC : (fc + 1) * FC],
                        start=(rt == 0),
                        stop=(rt == KT - 1),
                    )
                tt = t_pool.tile([P, FC], F32R, tag="t", name="tt")
                nc.vector.tensor_copy(tt[:], pt[:])
                nc.scalar.dma_start(
                    t_dram[cb * P : (cb + 1) * P, fc * FC : (fc + 1) * FC], tt[:]
                )

        # ---- pass 2: per f-block fb: Z[fb,:] = (T^T D)[fb,:]
        for fb in range(KT):
            tf = x_pool.tile([P, N], F32R, tag="x", name="tf")
            nc.sync.dma_start(
                tf[:].rearrange("p (t m) -> p t m", t=KT),
                t_dram[:, fb * P : (fb + 1) * P].rearrange("(t p) m -> p t m", p=P),
            )
            for g in range(NFC):
                pz = ps2.tile([P, FC], F32, tag="ps2", name="pz")
                for ct in range(KT):
                    nc.tensor.matmul(
                        pz[:],
                        lhsT=tf[:, ct * P : (ct + 1) * P],
                        rhs=d_sb[ct][:, g * FC : (g + 1) * FC],
                        start=(ct == 0),
                        stop=(ct == KT - 1),
                    )
                zt = z_pool.tile([P, FC], F32, tag="z", name="zt")
                nc.vector.tensor_copy(zt[:], pz[:])
                nc.scalar.dma_start(
                    z_ext[fb * P : (fb + 1) * P, g * FC : (g + 1) * FC], zt[:]
                )

    nc.finalize()
    return nc


def _build_bfly() -> bass.Bass:
    """Radix-2 even/odd DCT factorization in fp32r: each 1D DCT-II of size N
    becomes two size-N/2 cosine transforms of the folded sequences
    u = x_top + reverse(x_bot), v = x_top - reverse(x_bot):
        y[2j]   = sum_{n<H} u[n] De[n, j],   De[n,j] = cos(pi (2n+1) j / N)
        y[2j+1] = sum_{n<H} v[n] Do[n, j],   Do[n,j] = cos(pi (2n+1)(2j+1) / 2N)
    Halves the matmul work per pass.  Pass-1 folding is done on the host
    (u/v uploaded); pass-2 folding of the intermediate T is done by DVE with a
    reversed-row DMA load.  Outputs are de-interleaved on chip (strided DVE
    writes) + stride-2-row DMA stores, so all DRAM traffic stays contiguous
    per partition.
    """
    nc = bacc.Bacc(None, target_bir_lowering=False)
    u_ext = nc.declare_dram_parameter("u", [H, N], F32R, isOutput=False)
    v_ext = nc.declare_dram_parameter("v", [H, N], F32R, isOutput=False)
    de_ext = nc.declare_dram_parameter("de", [H, H], F32R, isOutput=False)
    do_ext = nc.declare_dram_parameter("do", [H, H], F32R, isOutput=False)
    z_ext = nc.declare_dram_parameter("z", [N, N], F32, isOutput=True)

    with ExitStack() as ctx:
        tc = ctx.enter_context(tile.TileContext(nc))
        d_pool = ctx.enter_context(tc.tile_pool(name="d", bufs=1))
        x_pool = ctx.enter_context(tc.tile_pool(name="x", bufs=4))
        t_pool = ctx.enter_context(tc.tile_pool(name="t", bufs=4))
        b_pool = ctx.enter_context(tc.tile_pool(name="b", bufs=4))
        z_pool = ctx.enter_context(tc.tile_pool(name="z", bufs=3))
        dram = ctx.enter_context(tc.tile_pool(name="dram", bufs=1, space="DRAM"))
        # PSUM: pass-1 accumulators 2x1 bank; pass-2 output chains share one
        # 6-slot pool (6 banks) so slow de-interleave copies never stall the
        # next chain. 8 banks total (no reversal matmuls anymore).
        ps1 = ctx.enter_context(tc.tile_pool(name="ps1", bufs=2, space="PSUM"))
        ps2 = ctx.enter_context(tc.tile_pool(name="ps2", bufs=6, space="PSUM"))

        # T in blocked layout: cols [0,H) = even outputs, [H,2H) = odd
        t_dram = dram.tile([N, N], F32R, name="t_dram")

        de_sb = [
            d_pool.tile([P, H], F32R, tag=f"de{t}", name=f"de{t}")
            for t in range(KT2)
        ]
        do_sb = [
            d_pool.tile([P, H], F32R, tag=f"do{t}", name=f"do{t}")
            for t in range(KT2)
        ]

        def load_block(ext, cb, tag):
            w = x_pool.tile([P, H], F32R, tag=tag, name="w_" + tag)
            nc.sync.dma_start(
                w[:].rearrange("p (t m) -> p t m", t=KT2),
                ext[:, cb * P : (cb + 1) * P].rearrange("(t p) m -> p t m", p=P),
            )
            return w

        # loads in exact first-consumption order: u0, de jc0, de jc1, v0,
        # do jc0, do jc1; the pass-2 reversal matrix r last.
        u0 = load_block(u_ext, 0, "u")
        for jc in range(2):
            for t in range(KT2):
                nc.sync.dma_start(
                    de_sb[t][:, jc * FC : (jc + 1) * FC],
                    de_ext[t * P : (t + 1) * P, jc * FC : (jc + 1) * FC],
                )
        v0 = load_block(v_ext, 0, "v")
        for jc in range(2):
            for t in range(KT2):
                nc.sync.dma_start(
                    do_sb[t][:, jc * FC : (jc + 1) * FC],
                    do_ext[t * P : (t + 1) * P, jc * FC : (jc + 1) * FC],
                )

        # ---- pass 1: T_blk[cb, :] ----
        for cb in range(KT):
            ut = u0 if cb == 0 else load_block(u_ext, cb, "u")
            vt = v0 if cb == 0 else load_block(v_ext, cb, "v")
            for half, (wt, dsb) in enumerate(((ut, de_sb), (vt, do_sb))):
                for jc in range(2):
                    pt = ps1.tile([P, FC], F32, tag="acc", name="pt")
                    for rt in range(KT2):
                        nc.tensor.matmul(
                            pt[:],
                            lhsT=wt[:, rt * P : (rt + 1) * P],
                            rhs=dsb[rt][:, jc * FC : (jc + 1) * FC],
                            start=(rt == 0),
                            stop=(rt == KT2 - 1),
                        )
                    tt = t_pool.tile([P, FC], F32R, tag="t", name="tt")
                    nc.vector.tensor_copy(tt[:], pt[:])
                    col0 = half * H + jc * FC
                    # Bottom-half blocks (cb>=8) arrive partition-reversed
                    # (host reversed their lhsT columns) and are stored
                    # mirror-ordered: row 1024+k holds T[2047-k].  Then the
                    # pass-2 fold reads both halves with plain ascending loads.
                    row0 = cb * P if cb < KT2 else (23 * P - cb * P)
                    nc.scalar.dma_start(
                        t_dram[row0 : row0 + P, col0 : col0 + FC], tt[:]
                    )

        # ---- pass 2: fold T over rows, transform, de-interleave out ----
        # bot_rev[c', f] = T[2047-c', f]: partition reversal via one PE matmul
        # with the reversal permutation R (out[m,n] = bot[127-m, n]); the
        # tile-order flip (ct -> 7-ct) via a reversed free-dim view in the add.
        # Software-pipelined: loads run 3 blocks ahead, reversal matmul + DVE
        # fold 2 ahead, so block fb's chains never wait on its fold.
        folded: dict = {}

        def p2_load(fb):
            top = b_pool.tile([P, H], F32R, tag="top", name="top")
            nc.sync.dma_start(
                top[:].rearrange("p (t m) -> p t m", t=KT2),
                t_dram[0:H, fb * P : (fb + 1) * P].rearrange(
                    "(t p) m -> p t m", p=P
                ),
            )
            bot = b_pool.tile([P, H], F32R, tag="bot", name="bot")
            nc.sync.dma_start(
                bot[:].rearrange("p (t m) -> p t m", t=KT2),
                t_dram[H:N, fb * P : (fb + 1) * P].rearrange(
                    "(t p) m -> p t m", p=P
                ),
            )
            folded[fb] = (top, bot)

        def p2_fold(fb):
            # mirror-ordered bottom storage makes the fold a plain 2D add/sub
            top, bot = folded[fb]
            u2 = b_pool.tile([P, H], F32R, tag="u2", name="u2")
            nc.vector.tensor_add(u2[:], top[:], bot[:])
            v2 = b_pool.tile([P, H], F32R, tag="v2", name="v2")
            nc.vector.tensor_sub(v2[:], top[:], bot[:])
            folded[fb] = (u2, v2)

        p2_load(0)
        p2_load(1)
        p2_fold(0)
        p2_load(2)
        p2_fold(1)
        for fb in range(KT):
            u2, v2 = folded.pop(fb)
            # f_blk block fb -> actual Z rows (de-interleave rows via stride 2)
            if fb < KT2:
                row0 = 2 * fb * P
                row_stop = row0 + 2 * P
            else:
                row0 = 2 * (fb - KT2) * P + 1
                row_stop = row0 + 2 * P - 1
            for jc in range(2):
                pe_ = ps2.tile([P, FC], F32, tag="zacc", name="pe_")
                for ct in range(KT2):
                    nc.tensor.matmul(
                        pe_[:],
                        lhsT=u2[:, ct * P : (ct + 1) * P],
                        rhs=de_sb[ct][:, jc * FC : (jc + 1) * FC],
                        start=(ct == 0),
                        stop=(ct == KT2 - 1),
                    )
                po_ = ps2.tile([P, FC], F32, tag="zacc", name="po_")
                for ct in range(KT2):
                    nc.tensor.matmul(
                        po_[:],
                        lhsT=v2[:, ct * P : (ct + 1) * P],
                        rhs=do_sb[ct][:, jc * FC : (jc + 1) * FC],
                        start=(ct == 0),
                        stop=(ct == KT2 - 1),
                    )
                zt = z_pool.tile([P, 2 * FC], F32, tag="z", name="zt")
                nc.scalar.copy(zt[:, 0 : 2 * FC : 2], pe_[:])
                nc.vector.tensor_copy(zt[:, 1 : 2 * FC : 2], po_[:])
                nc.scalar.dma_start(
                    z_ext[row0:row_stop:2, jc * 2 * FC : (jc + 1) * 2 * FC],
                    zt[:],
                )
            if fb + 3 < KT:
                p2_load(fb + 3)
            if fb + 2 < KT:
                p2_fold(fb + 2)

    nc.finalize()
    return nc


def _build_split() -> bass.Bass:
    """hi/lo bf16 decomposition: each logical matmul = 3 bf16 matmuls
    (Xh Dh + Xh Dl + Xl Dh), accumulated in the same PSUM chain."""
    nc = bacc.Bacc(None, target_bir_lowering=False)
    xh_ext = nc.declare_dram_parameter("xh", [N, N], BF16, isOutput=False)
    xl_ext = nc.declare_dram_parameter("xl", [N, N], BF16, isOutput=False)
    dh_ext = nc.declare_dram_parameter("dh", [N, N], BF16, isOutput=False)
    dl_ext = nc.declare_dram_parameter("dl", [N, N], BF16, isOutput=False)
    z_ext = nc.declare_dram_parameter("z", [N, N], F32, isOutput=True)

    with ExitStack() as ctx:
        tc = ctx.enter_context(tile.TileContext(nc))
        d_pool = ctx.enter_context(tc.tile_pool(name="d", bufs=1))
        x_pool = ctx.enter_context(tc.tile_pool(name="x", bufs=3))
        w_pool = ctx.enter_context(tc.tile_pool(name="w", bufs=3))
        t_pool = ctx.enter_context(tc.tile_pool(name="t", bufs=KT))
        z_pool = ctx.enter_context(tc.tile_pool(name="z", bufs=3))
        ps1 = ctx.enter_context(tc.tile_pool(name="ps1", bufs=4, space="PSUM"))
        ps2 = ctx.enter_context(tc.tile_pool(name="ps2", bufs=4, space="PSUM"))

        dh_sb = [
            d_pool.tile([P, N], BF16, tag=f"dh{t}", name=f"dh{t}")
            for t in range(KT)
        ]
        dl_sb = [
            d_pool.tile([P, N], BF16, tag=f"dl{t}", name=f"dl{t}")
            for t in range(KT)
        ]
        for fcol in range(NFC):
            for t in range(KT):
                nc.sync.dma_start(
                    dh_sb[t][:, fcol * FC : (fcol + 1) * FC],
                    dh_ext[t * P : (t + 1) * P, fcol * FC : (fcol + 1) * FC],
                )
                nc.sync.dma_start(
                    dl_sb[t][:, fcol * FC : (fcol + 1) * FC],
                    dl_ext[t * P : (t + 1) * P, fcol * FC : (fcol + 1) * FC],
                )

        for fc in range(NFC):
            t_tiles = []
            for cb in range(KT):
                xht = x_pool.tile([P, N], BF16, tag="xh", name="xht")
                xlt = x_pool.tile([P, N], BF16, tag="xl", name="xlt")
                for t_, ext in ((xht, xh_ext), (xlt, xl_ext)):
                    nc.sync.dma_start(
                        t_[:].rearrange("p (t m) -> p t m", t=KT),
                        ext[:, cb * P : (cb + 1) * P].rearrange(
                            "(t p) m -> p t m", p=P
                        ),
                    )
                pt = ps1.tile([P, FC], F32, tag="ps1", name="pt")
                nmm = 3 * KT
                i = 0
                for rt in range(KT):
                    dh = dh_sb[rt][:, fc * FC : (fc + 1) * FC]
                    dl = dl_sb[rt][:, fc * FC : (fc + 1) * FC]
                    xh = xht[:, rt * P : (rt + 1) * P]
                    xl = xlt[:, rt * P : (rt + 1) * P]
                    for l_, r_ in ((xh, dh), (xh, dl), (xl, dh)):
                        nc.tensor.matmul(
                            pt[:], lhsT=l_, rhs=r_,
                            start=(i == 0), stop=(i == nmm - 1),
                        )
                        i += 1
                # split T on device: th = bf16(T), tl = bf16(T - th)
                th = t_pool.tile([P, FC], BF16, tag="th", name="th")
                tl = t_pool.tile([P, FC], BF16, tag="tl", name="tl")
                tmp = w_pool.tile([P, FC], F32, tag="tmp", name="tmp")
                nc.vector.tensor_copy(th[:], pt[:])
                nc.scalar.copy(tmp[:], th[:])
                nc.vector.tensor_sub(tmp[:], pt[:], tmp[:])
                nc.vector.tensor_copy(tl[:], tmp[:])
                t_tiles.append((th, tl))

            for fb in range(FC // P):
                for g in range(NFC):
                    pz = ps2.tile([P, FC], F32, tag="ps2", name="pz")
                    nmm = 3 * KT
                    i = 0
                    for ct in range(KT):
                        th, tl = t_tiles[ct]
                        dh = dh_sb[ct][:, g * FC : (g + 1) * FC]
                        dl = dl_sb[ct][:, g * FC : (g + 1) * FC]
                        thb = th[:, fb * P : (fb + 1) * P]
                        tlb = tl[:, fb * P : (fb + 1) * P]
                        for l_, r_ in ((thb, dh), (thb, dl), (tlb, dh)):
                            nc.tensor.matmul(
                                pz[:], lhsT=l_, rhs=r_,
                                start=(i == 0), stop=(i == nmm - 1),
                            )
                            i += 1
                    zt = z_pool.tile([P, FC], F32, tag="z", name="zt")
                    nc.vector.tensor_copy(zt[:], pz[:])
                    row0 = (fc * (FC // P) + fb) * P
                    nc.sync.dma_start(
                        z_ext[row0 : row0 + P, g * FC : (g + 1) * FC], zt[:]
                    )

    nc.finalize()
    return nc


_PROGRAM_CACHE: dict = {}


_BUILDERS = {"f32r": _build_f32r, "bfly": _build_bfly, "split": _build_split}


def _get_program(mode: str) -> bass.Bass:
    if mode not in _PROGRAM_CACHE:
        _PROGRAM_CACHE[mode] = _BUILDERS[mode]()
    return _PROGRAM_CACHE[mode]


def _make_in_maps(x: np.ndarray, mode: str):
    if mode == "f32r":
        dr = _round_f32r(_dct_matrix_d())
        return [{"x": _round_f32r(x[i]), "d": dr} for i in range(B)]
    if mode == "bfly":
        n2 = np.arange(H, dtype=np.float64)[:, None]
        j2 = np.arange(H, dtype=np.float64)[None, :]
        de = _round_f32r(np.cos(np.pi * (2 * n2 + 1) * j2 / N).astype(np.float32))
        do = _round_f32r(
            np.cos(np.pi * (2 * n2 + 1) * (2 * j2 + 1) / (2 * N)).astype(
                np.float32
            )
        )
        maps = []
        for i in range(B):
            xi = np.asarray(x[i], dtype=np.float32)
            xr = xi[::-1]
            u = _round_f32r(xi[:H] + xr[:H])
            v = _round_f32r(xi[:H] - xr[:H])
            # Column-reverse blocks 8..15: pass-1 output partitions for those
            # blocks then come out mirror-ordered, which makes the pass-2
            # bottom-half fold a plain ascending read (see _build_bfly).
            for w in (u, v):
                blk = w[:, H:].reshape(H, KT2, P)
                w[:, H:] = blk[:, :, ::-1].reshape(H, H)
            maps.append({"u": u, "v": v, "de": de, "do": do})
        return maps
    dh, dl = _split_bf16(_dct_matrix_d())
    maps = []
    for i in range(B):
        xh, xl = _split_bf16(np.ascontiguousarray(x[i], dtype=np.float32))
        maps.append({"xh": xh, "xl": xl, "dh": dh, "dl": dl})
    return maps


def kernel(x: np.ndarray) -> np.ndarray:
    x = np.asarray(x)
    assert x.shape == (B, N, N), x.shape
    nc = _get_program(MODE)
    in_maps = _make_in_maps(x, MODE)
    res = run_bass_kernel_spmd(nc, in_maps, list(range(B)))
    out = np.stack([res.results[i]["z"] for i in range(B)], axis=0)
    return out.astype(np.float32, copy=False)



# revision 2
# speedup vs baseline: 1.5158x; 1.5158x over previous
"""Trainium2 Bass kernel: batched 2D DCT-II (unnormalized), x: (8, 2048, 2048) f32.

Math: per image X, the unnormalized 2D DCT-II is Z = C @ X @ C^T with
C[k,n] = cos(pi*(2n+1)*k/(2N)).  Two matmul passes (T = X^T D, Z = T^T D)
with the radix-2 even/odd cosine butterfly applied TWICE per pass:
    DCT-II_2048 -> DCT-II_1024 (even) + DCT-IV_1024 (odd)
    DCT-II_1024 -> DCT-II_512  (even) + DCT-IV_512  (odd)
so each 1D transform is three small matmuls (contractions 512/512/1024
against DII512/DIV512/DIV1024) = 0.75x the level-1 matmul work.

Sharding: batch dim 8 -> one image per NeuronCore (data parallel, no comms).

Device dataflow (mode "bfly2", fp16):
  - Host folds the input twice (uu, uv, v), bakes all mirror/permutation
    bookkeeping into the uploaded operands and cosine matrices, fp16.
  - Pass 1 streams uu/uv/v column-blocks; each c-block chain produces a
    [128, 2048] T row-block (frequency-blocked columns) kept in SBUF (fp16,
    no DRAM round-trip).  c-blocks run in mirror pairs so the pass-2 fold
    (plain DVE add/sub thanks to mirror storage) runs eagerly behind the PE.
  - Pass 2 contracts the folded arrays against the same cosine matrices and
    writes Z in fully blocked frequency order (plain contiguous DMA, fp16).
  - Host un-permutes rows/cols and casts to f32.

Modes: "bfly2" (default, fp16 level-2) and "bfly" (fp32r level-1 baseline).
"""

import numpy as np
from contextlib import ExitStack

import concourse.bass as bass
import concourse.bacc as bacc
import concourse.tile as tile
from concourse import mybir
from concourse.bass_utils import run_bass_kernel_spmd

F32 = mybir.dt.float32
F32R = mybir.dt.float32r
F16 = mybir.dt.float16

import os

MODE = os.environ.get("DCT_MODE", "bfly2")

B = 8          # batch == n_cores
N = 2048       # image is N x N
P = 128        # partitions
KT = N // P    # 16 k-tiles along a full contraction
FC = 512       # chunk width (PSUM bank = 512 f32)
NFC = N // FC
H = N // 2     # level-1 half (1024)
Q = N // 4     # level-2 quarter (512)
KT2 = H // P   # 8 k-tiles at half contraction
KT4 = Q // P   # 4 k-tiles at quarter contraction

# pass-1 chain order: mirror pairs so folds run eagerly.
# chain cb -> TT slot: A: cb 0-3 -> cb; B: cb 4-7 -> 11-cb; C: cb 8-11 -> cb;
# D: cb 12-15 -> 27-cb.  Pair (j, 15-j) completes (TT[j], TT[12+j]) -> fold
# s1[j], v2t[j]; pair (7-j, 8+j) completes (TT[4+j], TT[8+j]) -> fold s2[j],
# v2b[j]; both -> uu2[j], uv2[j].
PAIR_ORDER = [(0, 15), (7, 8), (1, 14), (6, 9), (2, 13), (5, 10), (3, 12), (4, 11)]


def _tt_slot(cb: int) -> int:
    if cb < 4:
        return cb
    if cb < 8:
        return 11 - cb
    if cb < 12:
        return cb
    return 27 - cb


def _round_f32r(a: np.ndarray) -> np.ndarray:
    b = np.ascontiguousarray(a, dtype=np.float32).view(np.uint32)
    return ((b + np.uint32(0x800)) & np.uint32(0xFFFFF000)).view(np.float32)


def _dct_mats_f64():
    n = np.arange(Q, dtype=np.float64)[:, None]
    j = np.arange(Q, dtype=np.float64)[None, :]
    dii512 = np.cos(np.pi * (2 * n + 1) * j / (2 * Q))
    div512 = np.cos(np.pi * (2 * n + 1) * (2 * j + 1) / (4 * Q))
    n = np.arange(H, dtype=np.float64)[:, None]
    j = np.arange(H, dtype=np.float64)[None, :]
    div1024 = np.cos(np.pi * (2 * n + 1) * (2 * j + 1) / (4 * H))
    return dii512, div512, div1024


# v (and DIV1024) row order: k-tiles 0-3 ascending (c' 0..511), k-tile 4+j
# holds rows 1023 - j*128 - p (mirrored bottom half, matching the v2b folds).
_VROWS = np.concatenate(
    [np.arange(Q)] + [1023 - j * P - np.arange(P) for j in range(4)]
)


def _build_bfly2() -> bass.Bass:
    nc = bacc.Bacc(None, target_bir_lowering=False)
    uu_ext = nc.declare_dram_parameter("uu", [Q, N], F16, isOutput=False)
    uv_ext = nc.declare_dram_parameter("uv", [Q, N], F16, isOutput=False)
    v_ext = nc.declare_dram_parameter("v", [H, N], F16, isOutput=False)
    dii_ext = nc.declare_dram_parameter("dii", [Q, Q], F16, isOutput=False)
    div_ext = nc.declare_dram_parameter("div", [Q, Q], F16, isOutput=False)
    dv2_ext = nc.declare_dram_parameter("dv2", [H, H], F16, isOutput=False)
    z_ext = nc.declare_dram_parameter("z", [N, N], F16, isOutput=True)

    with ExitStack() as ctx:
        tc = ctx.enter_context(tile.TileContext(nc))
        d_pool = ctx.enter_context(tc.tile_pool(name="d", bufs=1))
        in_pool = ctx.enter_context(tc.tile_pool(name="in", bufs=3))
        tt_pool = ctx.enter_context(tc.tile_pool(name="tt", bufs=5))
        fold_pool = ctx.enter_context(tc.tile_pool(name="fold", bufs=1))
        s_pool = ctx.enter_context(tc.tile_pool(name="s", bufs=2))
        z_pool = ctx.enter_context(tc.tile_pool(name="z", bufs=3))
        ps = ctx.enter_context(tc.tile_pool(name="ps", bufs=2, space="PSUM"))

        dii_sb = [d_pool.tile([P, Q], F16, tag=f"dii{t}", name=f"dii{t}") for t in range(KT4)]
        div_sb = [d_pool.tile([P, Q], F16, tag=f"div{t}", name=f"div{t}") for t in range(KT4)]
        dv2_sb = [d_pool.tile([P, H], F16, tag=f"dv{t}", name=f"dv{t}") for t in range(KT2)]

        def load_in(ext, cb, tag, nkt):
            w = in_pool.tile([P, nkt * P], F16, tag=tag, name="w_" + tag)
            nc.sync.dma_start(
                w[:].rearrange("p (t m) -> p t m", t=nkt),
                ext[:, cb * P : (cb + 1) * P].rearrange("(t p) m -> p t m", p=P),
            )
            return w

        # First-consumption-order loads: uu block, DII512, uv block, DIV512,
        # v block, DIV1024.
        cb0 = PAIR_ORDER[0][0]
        w0 = [load_in(uu_ext, cb0, "uu", KT4)]
        for t in range(KT4):
            nc.sync.dma_start(dii_sb[t][:], dii_ext[t * P : (t + 1) * P, :])
        w0.append(load_in(uv_ext, cb0, "uv", KT4))
        for t in range(KT4):
            nc.sync.dma_start(div_sb[t][:], div_ext[t * P : (t + 1) * P, :])
        w0.append(load_in(v_ext, cb0, "v", KT2))
        for t in range(KT2):
            nc.sync.dma_start(dv2_sb[t][:], dv2_ext[t * P : (t + 1) * P, :])

        # persistent fold tiles
        uu2 = [fold_pool.tile([P, N], F16, tag=f"uu2_{j}", name=f"uu2_{j}") for j in range(4)]
        uv2 = [fold_pool.tile([P, N], F16, tag=f"uv2_{j}", name=f"uv2_{j}") for j in range(4)]
        v2 = [fold_pool.tile([P, N], F16, tag=f"v2_{t}", name=f"v2_{t}") for t in range(KT2)]

        TT: dict = {}
        s1: dict = {}
        s2: dict = {}

        chain_order = [c for pair in PAIR_ORDER for c in pair]

        def p1_chain(ci, cb):
            if ci == 0:
                w_uu, w_uv, w_v = w0
            else:
                w_uu = load_in(uu_ext, cb, "uu", KT4)
                w_uv = load_in(uv_ext, cb, "uv", KT4)
                w_v = load_in(v_ext, cb, "v", KT2)
            p_uu = ps.tile([P, FC], F32, tag="a", name="p_uu")
            for rt in range(KT4):
                nc.tensor.matmul(
                    p_uu[:], lhsT=w_uu[:, rt * P : (rt + 1) * P], rhs=dii_sb[rt][:],
                    start=(rt == 0), stop=(rt == KT4 - 1),
                )
            p_uv = ps.tile([P, FC], F32, tag="b", name="p_uv")
            for rt in range(KT4):
                nc.tensor.matmul(
                    p_uv[:], lhsT=w_uv[:, rt * P : (rt + 1) * P], rhs=div_sb[rt][:],
                    start=(rt == 0), stop=(rt == KT4 - 1),
                )
            p_v0 = ps.tile([P, FC], F32, tag="c", name="p_v0")
            for rt in range(KT2):
                nc.tensor.matmul(
                    p_v0[:], lhsT=w_v[:, rt * P : (rt + 1) * P], rhs=dv2_sb[rt][:, 0:FC],
                    start=(rt == 0), stop=(rt == KT2 - 1),
                )
            p_v1 = ps.tile([P, FC], F32, tag="d", name="p_v1")
            for rt in range(KT2):
                nc.tensor.matmul(
                    p_v1[:], lhsT=w_v[:, rt * P : (rt + 1) * P], rhs=dv2_sb[rt][:, FC:H],
                    start=(rt == 0), stop=(rt == KT2 - 1),
                )
            tt = tt_pool.tile([P, N], F16, tag="tt", name="tt")
            nc.scalar.copy(tt[:, 0:FC], p_uu[:])
            nc.scalar.copy(tt[:, FC : 2 * FC], p_uv[:])
            nc.vector.tensor_copy(tt[:, 2 * FC : 3 * FC], p_v0[:])
            nc.vector.tensor_copy(tt[:, 3 * FC : N], p_v1[:])
            TT[_tt_slot(cb)] = tt

        def p1_fold(pi):
            j = pi // 2
            if pi % 2 == 0:  # A/D pair: TT[j], TT[12+j]
                a, d = TT.pop(j), TT.pop(12 + j)
                s = s_pool.tile([P, N], F16, tag="s1", name="s1")
                nc.vector.tensor_add(s[:], a[:], d[:])
                nc.vector.tensor_sub(v2[j][:], a[:], d[:])
                s1[j] = s
            else:  # B/C pair: TT[4+j], TT[8+j]
                b_, c_ = TT.pop(4 + j), TT.pop(8 + j)
                s = s_pool.tile([P, N], F16, tag="s2", name="s2")
                nc.vector.tensor_add(s[:], b_[:], c_[:])
                nc.vector.tensor_sub(v2[4 + j][:], b_[:], c_[:])
                s2[j] = s
                nc.vector.tensor_add(uu2[j][:], s1[j][:], s[:])
                nc.vector.tensor_sub(uv2[j][:], s1[j][:], s[:])

        for ci, cb in enumerate(chain_order):
            p1_chain(ci, cb)
            if ci % 2 == 1:
                p1_fold(ci // 2)

        # ---- pass 2: per f-block fb, Z row-block (blocked freq cols) ----
        for fb in range(KT):
            c0, c1 = fb * P, (fb + 1) * P
            p_e = ps.tile([P, FC], F32, tag="a", name="p_e")
            for ct in range(KT4):
                nc.tensor.matmul(
                    p_e[:], lhsT=uu2[ct][:, c0:c1], rhs=dii_sb[ct][:],
                    start=(ct == 0), stop=(ct == KT4 - 1),
                )
            p_m = ps.tile([P, FC], F32, tag="b", name="p_m")
            for ct in range(KT4):
                nc.tensor.matmul(
                    p_m[:], lhsT=uv2[ct][:, c0:c1], rhs=div_sb[ct][:],
                    start=(ct == 0), stop=(ct == KT4 - 1),
                )
            p_o0 = ps.tile([P, FC], F32, tag="c", name="p_o0")
            for ct in range(KT2):
                nc.tensor.matmul(
                    p_o0[:], lhsT=v2[ct][:, c0:c1], rhs=dv2_sb[ct][:, 0:FC],
                    start=(ct == 0), stop=(ct == KT2 - 1),
                )
            p_o1 = ps.tile([P, FC], F32, tag="d", name="p_o1")
            for ct in range(KT2):
                nc.tensor.matmul(
                    p_o1[:], lhsT=v2[ct][:, c0:c1], rhs=dv2_sb[ct][:, FC:H],
                    start=(ct == 0), stop=(ct == KT2 - 1),
                )
            zt = z_pool.tile([P, N], F16, tag="z", name="zt")
            nc.scalar.copy(zt[:, 0:FC], p_e[:])
            nc.scalar.copy(zt[:, FC : 2 * FC], p_m[:])
            nc.vector.tensor_copy(zt[:, 2 * FC : 3 * FC], p_o0[:])
            nc.vector.tensor_copy(zt[:, 3 * FC : N], p_o1[:])
            nc.scalar.dma_start(z_ext[c0:c1, :], zt[:])

    nc.finalize()
    return nc


def _build_bfly() -> bass.Bass:
    """Level-1 fp32r butterfly with T round-tripped via DRAM (baseline)."""
    nc = bacc.Bacc(None, target_bir_lowering=False)
    u_ext = nc.declare_dram_parameter("u", [H, N], F32R, isOutput=False)
    v_ext = nc.declare_dram_parameter("v", [H, N], F32R, isOutput=False)
    de_ext = nc.declare_dram_parameter("de", [H, H], F32R, isOutput=False)
    do_ext = nc.declare_dram_parameter("do", [H, H], F32R, isOutput=False)
    z_ext = nc.declare_dram_parameter("z", [N, N], F32, isOutput=True)

    with ExitStack() as ctx:
        tc = ctx.enter_context(tile.TileContext(nc))
        d_pool = ctx.enter_context(tc.tile_pool(name="d", bufs=1))
        x_pool = ctx.enter_context(tc.tile_pool(name="x", bufs=4))
        t_pool = ctx.enter_context(tc.tile_pool(name="t", bufs=4))
        b_pool = ctx.enter_context(tc.tile_pool(name="b", bufs=4))
        z_pool = ctx.enter_context(tc.tile_pool(name="z", bufs=3))
        dram = ctx.enter_context(tc.tile_pool(name="dram", bufs=1, space="DRAM"))
        ps1 = ctx.enter_context(tc.tile_pool(name="ps1", bufs=2, space="PSUM"))
        ps2 = ctx.enter_context(tc.tile_pool(name="ps2", bufs=6, space="PSUM"))

        t_dram = dram.tile([N, N], F32R, name="t_dram")

        de_sb = [d_pool.tile([P, H], F32R, tag=f"de{t}", name=f"de{t}") for t in range(KT2)]
        do_sb = [d_pool.tile([P, H], F32R, tag=f"do{t}", name=f"do{t}") for t in range(KT2)]

        def load_block(ext, cb, tag):
            w = x_pool.tile([P, H], F32R, tag=tag, name="w_" + tag)
            nc.sync.dma_start(
                w[:].rearrange("p (t m) -> p t m", t=KT2),
                ext[:, cb * P : (cb + 1) * P].rearrange("(t p) m -> p t m", p=P),
            )
            return w

        u0 = load_block(u_ext, 0, "u")
        for jc in range(2):
            for t in range(KT2):
                nc.sync.dma_start(
                    de_sb[t][:, jc * FC : (jc + 1) * FC],
                    de_ext[t * P : (t + 1) * P, jc * FC : (jc + 1) * FC],
                )
        v0 = load_block(v_ext, 0, "v")
        for jc in range(2):
            for t in range(KT2):
                nc.sync.dma_start(
                    do_sb[t][:, jc * FC : (jc + 1) * FC],
                    do_ext[t * P : (t + 1) * P, jc * FC : (jc + 1) * FC],
                )

        for cb in range(KT):
            ut = u0 if cb == 0 else load_block(u_ext, cb, "u")
            vt = v0 if cb == 0 else load_block(v_ext, cb, "v")
            for half, (wt, dsb) in enumerate(((ut, de_sb), (vt, do_sb))):
                for jc in range(2):
                    pt = ps1.tile([P, FC], F32, tag="acc", name="pt")
                    for rt in range(KT2):
                        nc.tensor.matmul(
                            pt[:],
                            lhsT=wt[:, rt * P : (rt + 1) * P],
                            rhs=dsb[rt][:, jc * FC : (jc + 1) * FC],
                            start=(rt == 0),
                            stop=(rt == KT2 - 1),
                        )
                    tt = t_pool.tile([P, FC], F32R, tag="t", name="tt")
                    nc.vector.tensor_copy(tt[:], pt[:])
                    col0 = half * H + jc * FC
                    row0 = cb * P if cb < KT2 else (23 * P - cb * P)
                    nc.scalar.dma_start(t_dram[row0 : row0 + P, col0 : col0 + FC], tt[:])

        folded: dict = {}

        def p2_load(fb):
            top = b_pool.tile([P, H], F32R, tag="top", name="top")
            nc.sync.dma_start(
                top[:].rearrange("p (t m) -> p t m", t=KT2),
                t_dram[0:H, fb * P : (fb + 1) * P].rearrange("(t p) m -> p t m", p=P),
            )
            bot = b_pool.tile([P, H], F32R, tag="bot", name="bot")
            nc.sync.dma_start(
                bot[:].rearrange("p (t m) -> p t m", t=KT2),
                t_dram[H:N, fb * P : (fb + 1) * P].rearrange("(t p) m -> p t m", p=P),
            )
            folded[fb] = (top, bot)

        def p2_fold(fb):
            top, bot = folded[fb]
            u2 = b_pool.tile([P, H], F32R, tag="u2", name="u2")
            nc.vector.tensor_add(u2[:], top[:], bot[:])
            v2_ = b_pool.tile([P, H], F32R, tag="v2", name="v2")
            nc.vector.tensor_sub(v2_[:], top[:], bot[:])
            folded[fb] = (u2, v2_)

        p2_load(0)
        p2_load(1)
        p2_fold(0)
        p2_load(2)
        p2_fold(1)
        for fb in range(KT):
            u2, v2_ = folded.pop(fb)
            if fb < KT2:
                row0 = 2 * fb * P
                row_stop = row0 + 2 * P
            else:
                row0 = 2 * (fb - KT2) * P + 1
                row_stop = row0 + 2 * P - 1
            for jc in range(2):
                pe_ = ps2.tile([P, FC], F32, tag="zacc", name="pe_")
                for ct in range(KT2):
                    nc.tensor.matmul(
                        pe_[:],
                        lhsT=u2[:, ct * P : (ct + 1) * P],
                        rhs=de_sb[ct][:, jc * FC : (jc + 1) * FC],
                        start=(ct == 0),
                        stop=(ct == KT2 - 1),
                    )
                po_ = ps2.tile([P, FC], F32, tag="zacc", name="po_")
                for ct in range(KT2):
                    nc.tensor.matmul(
                        po_[:],
                        lhsT=v2_[:, ct * P : (ct + 1) * P],
                        rhs=do_sb[ct][:, jc * FC : (jc + 1) * FC],
                        start=(ct == 0),
                        stop=(ct == KT2 - 1),
                    )
                zt = z_pool.tile([P, 2 * FC], F32, tag="z", name="zt")
                nc.scalar.copy(zt[:, 0 : 2 * FC : 2], pe_[:])
                nc.vector.tensor_copy(zt[:, 1 : 2 * FC : 2], po_[:])
                nc.scalar.dma_start(
                    z_ext[row0:row_stop:2, jc * 2 * FC : (jc + 1) * 2 * FC], zt[:]
                )
            if fb + 3 < KT:
                p2_load(fb + 3)
            if fb + 2 < KT:
                p2_fold(fb + 2)

    nc.finalize()
    return nc


_PROGRAM_CACHE: dict = {}

_BUILDERS = {"bfly2": _build_bfly2, "bfly": _build_bfly}


def _get_program(mode: str) -> bass.Bass:
    if mode not in _PROGRAM_CACHE:
        _PROGRAM_CACHE[mode] = _BUILDERS[mode]()
    return _PROGRAM_CACHE[mode]


def _mirror_cols(a: np.ndarray) -> np.ndarray:
    """Reverse c-columns within blocks 4-7 and 12-15 (pass-1 chains for those
    blocks then emit mirror-ordered partitions, making pass-2 folds plain)."""
    a = a.copy()
    for cb in list(range(4, 8)) + list(range(12, 16)):
        blk = a[:, cb * P : (cb + 1) * P]
        a[:, cb * P : (cb + 1) * P] = blk[:, ::-1]
    return a


def _make_in_maps(x: np.ndarray, mode: str):
    if mode == "bfly2":
        dii512, div512, div1024 = _dct_mats_f64()
        dii = dii512.astype(np.float16)
        div = div512.astype(np.float16)
        dv2 = div1024[_VROWS].astype(np.float16)
        maps = []
        for i in range(B):
            xf = np.asarray(x[i], dtype=np.float64)
            u = xf[:H] + xf[N - 1 : H - 1 : -1]
            vv = xf[:H] - xf[N - 1 : H - 1 : -1]
            uu = u[:Q] + u[H - 1 : Q - 1 : -1]
            uv = u[:Q] - u[H - 1 : Q - 1 : -1]
            uu = _mirror_cols(uu).astype(np.float16)
            uv = _mirror_cols(uv).astype(np.float16)
            vv = _mirror_cols(vv)[_VROWS].astype(np.float16)
            maps.append({"uu": uu, "uv": uv, "v": vv, "dii": dii, "div": div, "dv2": dv2})
        return maps
    # level-1 fp32r baseline
    n2 = np.arange(H, dtype=np.float64)[:, None]
    j2 = np.arange(H, dtype=np.float64)[None, :]
    de = _round_f32r(np.cos(np.pi * (2 * n2 + 1) * j2 / N).astype(np.float32))
    do = _round_f32r(
        np.cos(np.pi * (2 * n2 + 1) * (2 * j2 + 1) / (2 * N)).astype(np.float32)
    )
    maps = []
    for i in range(B):
        xi = np.asarray(x[i], dtype=np.float32)
        xr = xi[::-1]
        u = _round_f32r(xi[:H] + xr[:H])
        v = _round_f32r(xi[:H] - xr[:H])
        for w in (u, v):
            blk = w[:, H:].reshape(H, KT2, P)
            w[:, H:] = blk[:, :, ::-1].reshape(H, H)
        maps.append({"u": u, "v": v, "de": de, "do": do})
    return maps


# blocked frequency index b -> actual frequency f
_FREQ = np.where(
    np.arange(N) < Q,
    4 * np.arange(N),
    np.where(np.arange(N) < H, 4 * (np.arange(N) - Q) + 2, 2 * (np.arange(N) - H) + 1),
)
_INV = np.empty(N, dtype=np.int64)
_INV[_FREQ] = np.arange(N)


def kernel(x: np.ndarray) -> np.ndarray:
    x = np.asarray(x)
    assert x.shape == (B, N, N), x.shape
    nc = _get_program(MODE)
    in_maps = _make_in_maps(x, MODE)
    res = run_bass_kernel_spmd(nc, in_maps, list(range(B)))
    if MODE == "bfly2":
        out = np.empty((B, N, N), dtype=np.float32)
        for i in range(B):
            zb = np.asarray(res.results[i]["z"]).astype(np.float32)
            out[i] = zb[_INV][:, _INV]
        return out
    out = np.stack([res.results[i]["z"] for i in range(B)], axis=0)
    return out.astype(np.float32, copy=False)


# revision 3
# speedup vs baseline: 1.6155x; 1.0658x over previous
"""Trainium2 Bass kernel: batched 2D DCT-II (unnormalized), x: (8, 2048, 2048) f32.

Math: per image X, the unnormalized 2D DCT-II is Z = C @ X @ C^T with
C[k,n] = cos(pi*(2n+1)*k/(2N)).  Two matmul passes (T = X^T D, Z = T^T D)
with the radix-2 even/odd cosine butterfly applied THREE times per pass:
    DCT-II_2048 -> DCT-II_1024 (even) + DCT-IV_1024 (odd)
    DCT-II_1024 -> DCT-II_512  (even) + DCT-IV_512  (odd)
    DCT-II_512  -> DCT-II_256  (even) + DCT-IV_256  (odd)
so each 1D transform contracts 256+256+512+1024 rows against the small
cosine matrices DII256/DIV256/DIV512/DIV1024 (0.69x the level-1 work).

Sharding: batch dim 8 -> one image per NeuronCore (data parallel, no comms).

Device dataflow (mode "bfly3", fp16):
  - Host folds the input three times and stacks everything into ONE upload
    array per image; each 128-row block of the intermediate T carries an
    orientation (ascending/descending, a consistent 2-coloring) baked into
    the uploaded operand columns and cosine-matrix rows, which makes EVERY
    fold level on device a plain partition-aligned DVE add/sub.
  - Pass 1 streams one fused column-block per chain; T row-blocks stay in
    SBUF (fp16, no DRAM round-trip).  Chains run in mirror-pair order so
    folds run eagerly behind the PE.
  - Pass 2 contracts the folded arrays against the same cosine matrices
    (k-tile order puts late-folded tiles last, hiding the pass boundary)
    and writes Z in fully blocked frequency order (plain contiguous DMA).
  - Host un-permutes rows/cols and casts fp16 -> f32.

Modes: "bfly3" (default, level-3) and "bfly2" (level-2, previous version).
"""

import numpy as np
from contextlib import ExitStack

import concourse.bass as bass
import concourse.bacc as bacc
import concourse.tile as tile
from concourse import mybir
from concourse.bass_utils import run_bass_kernel_spmd

F32 = mybir.dt.float32
F16 = mybir.dt.float16

import os

MODE = os.environ.get("DCT_MODE", "bfly3")

B = 8          # batch == n_cores
N = 2048       # image is N x N
P = 128        # partitions
KT = N // P    # 16
FC = 512       # PSUM bank width (f32)
H = N // 2     # 1024
Q = N // 4     # 512
E = N // 8     # 256
KT2 = H // P   # 8
KT4 = Q // P   # 4

# ---- orientation 2-coloring of the 16 T row-blocks (True = ascending) ----
ASC16 = [True, True, False, False, True, True, False, False,
         True, True, False, False, True, True, False, False]
ASC8 = ASC16[:8]
ASC4 = ASC8[:4]

# pass-1 chain order: mirror pairs, grouped so every fold level completes
# as early as possible (chain cb's output block IS T block cb, in ASC16[cb]
# orientation).
CHAIN_ORDER = [0, 15, 7, 8, 3, 12, 4, 11, 1, 14, 6, 9, 2, 13, 5, 10]
# after chain index ci (1-based position), run these folds:
#   level-1 tile j: u2[j]=TT[j]+TT[15-j], v2[j]=TT[j]-TT[15-j]
#   level-2 tile i: uu2[i]=u2[i]+u2[7-i], uv2[i]=u2[i]-u2[7-i]
#   level-3 tile m: uuu2[m]=uu2[m]+uu2[3-m], uuv2[m]=uu2[m]-uu2[3-m]
# pass-2 k-tile orders put the last-folded tiles at chain end:
KORD_V = [0, 7, 3, 4, 1, 6, 2, 5]
KORD_UV = [0, 3, 1, 2]


def _rowperm(nblk, asc):
    out = []
    for j in range(nblk):
        p = np.arange(P)
        out.append(j * P + (p if asc[j] else P - 1 - p))
    return np.concatenate(out)


ROWP1024 = _rowperm(8, ASC8)
ROWP512 = _rowperm(4, ASC4)


def _dct_mats_f64(M, kind):
    n = np.arange(M, dtype=np.float64)[:, None]
    j = np.arange(M, dtype=np.float64)[None, :]
    if kind == "II":
        return np.cos(np.pi * (2 * n + 1) * j / (2 * M))
    return np.cos(np.pi * (2 * n + 1) * (2 * j + 1) / (4 * M))


def _build_bfly3() -> bass.Bass:
    nc = bacc.Bacc(None, target_bir_lowering=False)
    w_ext = nc.declare_dram_parameter("w", [N, N], F16, isOutput=False)
    d256_ext = nc.declare_dram_parameter("d256", [Q, E], F16, isOutput=False)
    d512_ext = nc.declare_dram_parameter("d512", [Q, Q], F16, isOutput=False)
    dv2_ext = nc.declare_dram_parameter("dv2", [H, H], F16, isOutput=False)
    z_ext = nc.declare_dram_parameter("z", [N, N], F16, isOutput=True)

    with ExitStack() as ctx:
        tc = ctx.enter_context(tile.TileContext(nc))
        d_pool = ctx.enter_context(tc.tile_pool(name="d", bufs=1))
        in_pool = ctx.enter_context(tc.tile_pool(name="in", bufs=3))
        tt_pool = ctx.enter_context(tc.tile_pool(name="tt", bufs=5))
        f1_pool = ctx.enter_context(tc.tile_pool(name="f1", bufs=4))
        f2_pool = ctx.enter_context(tc.tile_pool(name="f2", bufs=3))
        op_pool = ctx.enter_context(tc.tile_pool(name="op", bufs=1))
        z_pool = ctx.enter_context(tc.tile_pool(name="z", bufs=3))
        ps = ctx.enter_context(tc.tile_pool(name="ps", bufs=2, space="PSUM"))

        def load_w(cb):
            w = in_pool.tile([P, N], F16, tag="w", name="w")
            nc.sync.dma_start(
                w[:].rearrange("p (t m) -> p t m", t=KT),
                w_ext[:, cb * P : (cb + 1) * P].rearrange("(t p) m -> p t m", p=P),
            )
            return w

        # first chain's operand block, then the three cosine-matrix loads
        w_first = load_w(CHAIN_ORDER[0])
        d256_sb = d_pool.tile([P, 4 * E], F16, tag="d256", name="d256")
        nc.sync.dma_start(
            d256_sb[:].rearrange("p (t j) -> p t j", t=4),
            d256_ext[:].rearrange("(t p) j -> p t j", p=P),
        )
        d512_sb = d_pool.tile([P, 4 * Q], F16, tag="d512", name="d512")
        nc.sync.dma_start(
            d512_sb[:].rearrange("p (t j) -> p t j", t=4),
            d512_ext[:].rearrange("(t p) j -> p t j", p=P),
        )
        dv2_sb = d_pool.tile([P, 8 * H], F16, tag="dv2", name="dv2")
        nc.sync.dma_start(
            dv2_sb[:].rearrange("p (t j) -> p t j", t=8),
            dv2_ext[:].rearrange("(t p) j -> p t j", p=P),
        )

        # persistent matmul-operand fold tiles
        v2 = [op_pool.tile([P, N], F16, tag=f"v2_{j}", name=f"v2_{j}") for j in range(8)]
        uv2 = [op_pool.tile([P, N], F16, tag=f"uv2_{i}", name=f"uv2_{i}") for i in range(4)]
        uuu2 = [op_pool.tile([P, N], F16, tag=f"uuu2_{m}", name=f"uuu2_{m}") for m in range(2)]
        uuv2 = [op_pool.tile([P, N], F16, tag=f"uuv2_{m}", name=f"uuv2_{m}") for m in range(2)]

        TT: dict = {}
        u2: dict = {}
        uu2: dict = {}

        def p1_chain(ci, cb):
            w = w_first if ci == 0 else load_w(cb)
            p_a = ps.tile([P, FC], F32, tag="a", name="p_a")
            for t in range(2):  # uuu: k-tiles 0,1 -> psum cols 0:256
                nc.tensor.matmul(
                    p_a[:, 0:E], lhsT=w[:, t * P : (t + 1) * P],
                    rhs=d256_sb[:, t * E : (t + 1) * E],
                    start=(t == 0), stop=(t == 1),
                )
            for t in range(2):  # uuv: k-tiles 2,3 -> psum cols 256:512
                nc.tensor.matmul(
                    p_a[:, E:FC], lhsT=w[:, (2 + t) * P : (3 + t) * P],
                    rhs=d256_sb[:, (2 + t) * E : (3 + t) * E],
                    start=(t == 0), stop=(t == 1),
                )
            p_b = ps.tile([P, FC], F32, tag="b", name="p_b")
            for t in range(KT4):  # uv: k-tiles 4..7
                nc.tensor.matmul(
                    p_b[:], lhsT=w[:, (4 + t) * P : (5 + t) * P],
                    rhs=d512_sb[:, t * Q : (t + 1) * Q],
                    start=(t == 0), stop=(t == KT4 - 1),
                )
            p_c = ps.tile([P, FC], F32, tag="c", name="p_c")
            for t in range(KT2):  # v chunk 0: k-tiles 8..15
                nc.tensor.matmul(
                    p_c[:], lhsT=w[:, (8 + t) * P : (9 + t) * P],
                    rhs=dv2_sb[:, t * H : t * H + FC],
                    start=(t == 0), stop=(t == KT2 - 1),
                )
            p_d = ps.tile([P, FC], F32, tag="d", name="p_d")
            for t in range(KT2):  # v chunk 1
                nc.tensor.matmul(
                    p_d[:], lhsT=w[:, (8 + t) * P : (9 + t) * P],
                    rhs=dv2_sb[:, t * H + FC : (t + 1) * H],
                    start=(t == 0), stop=(t == KT2 - 1),
                )
            tt = tt_pool.tile([P, N], F16, tag="tt", name="tt")
            nc.scalar.copy(tt[:, 0:FC], p_a[:])
            nc.scalar.copy(tt[:, FC : 2 * FC], p_b[:])
            nc.vector.tensor_copy(tt[:, 2 * FC : 3 * FC], p_c[:])
            nc.vector.tensor_copy(tt[:, 3 * FC : N], p_d[:])
            TT[cb] = tt

        def fold1(j):
            a, b_ = TT.pop(j), TT.pop(15 - j)
            s = f1_pool.tile([P, N], F16, tag="u2", name="u2")
            nc.vector.tensor_add(s[:], a[:], b_[:])
            nc.vector.tensor_sub(v2[j][:], a[:], b_[:])
            u2[j] = s

        def fold2(i):
            a, b_ = u2.pop(i), u2.pop(7 - i)
            s = f2_pool.tile([P, N], F16, tag="uu2", name="uu2")
            nc.vector.tensor_add(s[:], a[:], b_[:])
            nc.vector.tensor_sub(uv2[i][:], a[:], b_[:])
            uu2[i] = s

        def fold3(m):
            a, b_ = uu2.pop(m), uu2.pop(3 - m)
            nc.vector.tensor_add(uuu2[m][:], a[:], b_[:])
            nc.vector.tensor_sub(uuv2[m][:], a[:], b_[:])

        FOLDS = {2: [("1", 0)], 4: [("1", 7), ("2", 0)],
                 6: [("1", 3)], 8: [("1", 4), ("2", 3), ("3", 0)],
                 10: [("1", 1)], 12: [("1", 6), ("2", 1)],
                 14: [("1", 2)], 16: [("1", 5), ("2", 2), ("3", 1)]}

        for ci, cb in enumerate(CHAIN_ORDER):
            p1_chain(ci, cb)
            for lvl, idx in FOLDS.get(ci + 1, []):
                (fold1 if lvl == "1" else fold2 if lvl == "2" else fold3)(idx)

        # ---- pass 2 ----
        for fb in range(KT):
            c0, c1 = fb * P, (fb + 1) * P
            p_a = ps.tile([P, FC], F32, tag="a", name="p_a")
            for n_, m in enumerate([0, 1]):
                nc.tensor.matmul(
                    p_a[:, 0:E], lhsT=uuu2[m][:, c0:c1],
                    rhs=d256_sb[:, m * E : (m + 1) * E],
                    start=(n_ == 0), stop=(n_ == 1),
                )
            for n_, m in enumerate([0, 1]):
                nc.tensor.matmul(
                    p_a[:, E:FC], lhsT=uuv2[m][:, c0:c1],
                    rhs=d256_sb[:, (2 + m) * E : (3 + m) * E],
                    start=(n_ == 0), stop=(n_ == 1),
                )
            p_b = ps.tile([P, FC], F32, tag="b", name="p_b")
            for n_, i in enumerate(KORD_UV):
                nc.tensor.matmul(
                    p_b[:], lhsT=uv2[i][:, c0:c1],
                    rhs=d512_sb[:, i * Q : (i + 1) * Q],
                    start=(n_ == 0), stop=(n_ == KT4 - 1),
                )
            p_c = ps.tile([P, FC], F32, tag="c", name="p_c")
            for n_, j in enumerate(KORD_V):
                nc.tensor.matmul(
                    p_c[:], lhsT=v2[j][:, c0:c1],
                    rhs=dv2_sb[:, j * H : j * H + FC],
                    start=(n_ == 0), stop=(n_ == KT2 - 1),
                )
            p_d = ps.tile([P, FC], F32, tag="d", name="p_d")
            for n_, j in enumerate(KORD_V):
                nc.tensor.matmul(
                    p_d[:], lhsT=v2[j][:, c0:c1],
                    rhs=dv2_sb[:, j * H + FC : (j + 1) * H],
                    start=(n_ == 0), stop=(n_ == KT2 - 1),
                )
            zt = z_pool.tile([P, N], F16, tag="z", name="zt")
            nc.scalar.copy(zt[:, 0:FC], p_a[:])
            nc.scalar.copy(zt[:, FC : 2 * FC], p_b[:])
            nc.vector.tensor_copy(zt[:, 2 * FC : 3 * FC], p_c[:])
            nc.vector.tensor_copy(zt[:, 3 * FC : N], p_d[:])
            nc.scalar.dma_start(z_ext[c0:c1, :], zt[:])

    nc.finalize()
    return nc


# ---------------- level-2 variant (previous version, fallback) ----------------

PAIR_ORDER2 = [(0, 15), (7, 8), (1, 14), (6, 9), (2, 13), (5, 10), (3, 12), (4, 11)]
_VROWS2 = np.concatenate(
    [np.arange(Q)] + [1023 - j * P - np.arange(P) for j in range(4)]
)


def _tt_slot2(cb: int) -> int:
    if cb < 4:
        return cb
    if cb < 8:
        return 11 - cb
    if cb < 12:
        return cb
    return 27 - cb


def _build_bfly2() -> bass.Bass:
    nc = bacc.Bacc(None, target_bir_lowering=False)
    uu_ext = nc.declare_dram_parameter("uu", [Q, N], F16, isOutput=False)
    uv_ext = nc.declare_dram_parameter("uv", [Q, N], F16, isOutput=False)
    v_ext = nc.declare_dram_parameter("v", [H, N], F16, isOutput=False)
    dii_ext = nc.declare_dram_parameter("dii", [Q, Q], F16, isOutput=False)
    div_ext = nc.declare_dram_parameter("div", [Q, Q], F16, isOutput=False)
    dv2_ext = nc.declare_dram_parameter("dv2", [H, H], F16, isOutput=False)
    z_ext = nc.declare_dram_parameter("z", [N, N], F16, isOutput=True)

    with ExitStack() as ctx:
        tc = ctx.enter_context(tile.TileContext(nc))
        d_pool = ctx.enter_context(tc.tile_pool(name="d", bufs=1))
        in_pool = ctx.enter_context(tc.tile_pool(name="in", bufs=3))
        tt_pool = ctx.enter_context(tc.tile_pool(name="tt", bufs=5))
        fold_pool = ctx.enter_context(tc.tile_pool(name="fold", bufs=1))
        s_pool = ctx.enter_context(tc.tile_pool(name="s", bufs=2))
        z_pool = ctx.enter_context(tc.tile_pool(name="z", bufs=3))
        ps = ctx.enter_context(tc.tile_pool(name="ps", bufs=2, space="PSUM"))

        dii_sb = [d_pool.tile([P, Q], F16, tag=f"dii{t}", name=f"dii{t}") for t in range(KT4)]
        div_sb = [d_pool.tile([P, Q], F16, tag=f"div{t}", name=f"div{t}") for t in range(KT4)]
        dv2_sb = [d_pool.tile([P, H], F16, tag=f"dv{t}", name=f"dv{t}") for t in range(KT2)]

        def load_in(ext, cb, tag, nkt):
            w = in_pool.tile([P, nkt * P], F16, tag=tag, name="w_" + tag)
            nc.sync.dma_start(
                w[:].rearrange("p (t m) -> p t m", t=nkt),
                ext[:, cb * P : (cb + 1) * P].rearrange("(t p) m -> p t m", p=P),
            )
            return w

        cb0 = PAIR_ORDER2[0][0]
        w0 = [load_in(uu_ext, cb0, "uu", KT4)]
        for t in range(KT4):
            nc.sync.dma_start(dii_sb[t][:], dii_ext[t * P : (t + 1) * P, :])
        w0.append(load_in(uv_ext, cb0, "uv", KT4))
        for t in range(KT4):
            nc.sync.dma_start(div_sb[t][:], div_ext[t * P : (t + 1) * P, :])
        w0.append(load_in(v_ext, cb0, "v", KT2))
        for t in range(KT2):
            nc.sync.dma_start(dv2_sb[t][:], dv2_ext[t * P : (t + 1) * P, :])

        uu2 = [fold_pool.tile([P, N], F16, tag=f"uu2_{j}", name=f"uu2_{j}") for j in range(4)]
        uv2 = [fold_pool.tile([P, N], F16, tag=f"uv2_{j}", name=f"uv2_{j}") for j in range(4)]
        v2 = [fold_pool.tile([P, N], F16, tag=f"v2_{t}", name=f"v2_{t}") for t in range(KT2)]

        TT: dict = {}
        s1: dict = {}
        s2: dict = {}

        chain_order = [c for pair in PAIR_ORDER2 for c in pair]

        def p1_chain(ci, cb):
            if ci == 0:
                w_uu, w_uv, w_v = w0
            else:
                w_uu = load_in(uu_ext, cb, "uu", KT4)
                w_uv = load_in(uv_ext, cb, "uv", KT4)
                w_v = load_in(v_ext, cb, "v", KT2)
            p_uu = ps.tile([P, FC], F32, tag="a", name="p_uu")
            for rt in range(KT4):
                nc.tensor.matmul(
                    p_uu[:], lhsT=w_uu[:, rt * P : (rt + 1) * P], rhs=dii_sb[rt][:],
                    start=(rt == 0), stop=(rt == KT4 - 1),
                )
            p_uv = ps.tile([P, FC], F32, tag="b", name="p_uv")
            for rt in range(KT4):
                nc.tensor.matmul(
                    p_uv[:], lhsT=w_uv[:, rt * P : (rt + 1) * P], rhs=div_sb[rt][:],
                    start=(rt == 0), stop=(rt == KT4 - 1),
                )
            p_v0 = ps.tile([P, FC], F32, tag="c", name="p_v0")
            for rt in range(KT2):
                nc.tensor.matmul(
                    p_v0[:], lhsT=w_v[:, rt * P : (rt + 1) * P], rhs=dv2_sb[rt][:, 0:FC],
                    start=(rt == 0), stop=(rt == KT2 - 1),
                )
            p_v1 = ps.tile([P, FC], F32, tag="d", name="p_v1")
            for rt in range(KT2):
                nc.tensor.matmul(
                    p_v1[:], lhsT=w_v[:, rt * P : (rt + 1) * P], rhs=dv2_sb[rt][:, FC:H],
                    start=(rt == 0), stop=(rt == KT2 - 1),
                )
            tt = tt_pool.tile([P, N], F16, tag="tt", name="tt")
            nc.scalar.copy(tt[:, 0:FC], p_uu[:])
            nc.scalar.copy(tt[:, FC : 2 * FC], p_uv[:])
            nc.vector.tensor_copy(tt[:, 2 * FC : 3 * FC], p_v0[:])
            nc.vector.tensor_copy(tt[:, 3 * FC : N], p_v1[:])
            TT[_tt_slot2(cb)] = tt

        def p1_fold(pi):
            j = pi // 2
            if pi % 2 == 0:
                a, d = TT.pop(j), TT.pop(12 + j)
                s = s_pool.tile([P, N], F16, tag="s1", name="s1")
                nc.vector.tensor_add(s[:], a[:], d[:])
                nc.vector.tensor_sub(v2[j][:], a[:], d[:])
                s1[j] = s
            else:
                b_, c_ = TT.pop(4 + j), TT.pop(8 + j)
                s = s_pool.tile([P, N], F16, tag="s2", name="s2")
                nc.vector.tensor_add(s[:], b_[:], c_[:])
                nc.vector.tensor_sub(v2[4 + j][:], b_[:], c_[:])
                s2[j] = s
                nc.vector.tensor_add(uu2[j][:], s1[j][:], s[:])
                nc.vector.tensor_sub(uv2[j][:], s1[j][:], s[:])

        for ci, cb in enumerate(chain_order):
            p1_chain(ci, cb)
            if ci % 2 == 1:
                p1_fold(ci // 2)

        for fb in range(KT):
            c0, c1 = fb * P, (fb + 1) * P
            p_e = ps.tile([P, FC], F32, tag="a", name="p_e")
            for ct in range(KT4):
                nc.tensor.matmul(
                    p_e[:], lhsT=uu2[ct][:, c0:c1], rhs=dii_sb[ct][:],
                    start=(ct == 0), stop=(ct == KT4 - 1),
                )
            p_m = ps.tile([P, FC], F32, tag="b", name="p_m")
            for ct in range(KT4):
                nc.tensor.matmul(
                    p_m[:], lhsT=uv2[ct][:, c0:c1], rhs=div_sb[ct][:],
                    start=(ct == 0), stop=(ct == KT4 - 1),
                )
            p_o0 = ps.tile([P, FC], F32, tag="c", name="p_o0")
            for ct in range(KT2):
                nc.tensor.matmul(
                    p_o0[:], lhsT=v2[ct][:, c0:c1], rhs=dv2_sb[ct][:, 0:FC],
                    start=(ct == 0), stop=(ct == KT2 - 1),
                )
            p_o1 = ps.tile([P, FC], F32, tag="d", name="p_o1")
            for ct in range(KT2):
                nc.tensor.matmul(
                    p_o1[:], lhsT=v2[ct][:, c0:c1], rhs=dv2_sb[ct][:, FC:H],
                    start=(ct == 0), stop=(ct == KT2 - 1),
                )
            zt = z_pool.tile([P, N], F16, tag="z", name="zt")
            nc.scalar.copy(zt[:, 0:FC], p_e[:])
            nc.scalar.copy(zt[:, FC : 2 * FC], p_m[:])
            nc.vector.tensor_copy(zt[:, 2 * FC : 3 * FC], p_o0[:])
            nc.vector.tensor_copy(zt[:, 3 * FC : N], p_o1[:])
            nc.scalar.dma_start(z_ext[c0:c1, :], zt[:])

    nc.finalize()
    return nc


_PROGRAM_CACHE: dict = {}

_BUILDERS = {"bfly3": _build_bfly3, "bfly2": _build_bfly2}


def _get_program(mode: str) -> bass.Bass:
    if mode not in _PROGRAM_CACHE:
        _PROGRAM_CACHE[mode] = _BUILDERS[mode]()
    return _PROGRAM_CACHE[mode]


def _mirror_cols(a: np.ndarray, desc_blocks) -> np.ndarray:
    a = a.copy()
    for cb in desc_blocks:
        a[:, cb * P : (cb + 1) * P] = a[:, cb * P : (cb + 1) * P][:, ::-1]
    return a


def _make_in_maps(x: np.ndarray, mode: str):
    if mode == "bfly3":
        d256 = np.vstack(
            [_dct_mats_f64(E, "II"), _dct_mats_f64(E, "IV")]
        ).astype(np.float16)
        d512 = _dct_mats_f64(Q, "IV")[ROWP512].astype(np.float16)
        dv2 = _dct_mats_f64(H, "IV")[ROWP1024].astype(np.float16)
        desc = [k for k in range(16) if not ASC16[k]]
        maps = []
        for i in range(B):
            xf = np.asarray(x[i], dtype=np.float64)
            u = xf[:H] + xf[N - 1 : H - 1 : -1]
            vv = xf[:H] - xf[N - 1 : H - 1 : -1]
            uu = u[:Q] + u[H - 1 : Q - 1 : -1]
            uv = u[:Q] - u[H - 1 : Q - 1 : -1]
            uuu = uu[:E] + uu[Q - 1 : E - 1 : -1]
            uuv = uu[:E] - uu[Q - 1 : E - 1 : -1]
            w_all = np.vstack([uuu, uuv, uv[ROWP512], vv[ROWP1024]])
            w_all = _mirror_cols(w_all, desc).astype(np.float16)
            maps.append({"w": w_all, "d256": d256, "d512": d512, "dv2": dv2})
        return maps
    # bfly2
    dii = _dct_mats_f64(Q, "II").astype(np.float16)
    div = _dct_mats_f64(Q, "IV").astype(np.float16)
    dv2 = _dct_mats_f64(H, "IV")[_VROWS2].astype(np.float16)
    desc = list(range(4, 8)) + list(range(12, 16))
    maps = []
    for i in range(B):
        xf = np.asarray(x[i], dtype=np.float64)
        u = xf[:H] + xf[N - 1 : H - 1 : -1]
        vv = xf[:H] - xf[N - 1 : H - 1 : -1]
        uu = u[:Q] + u[H - 1 : Q - 1 : -1]
        uv = u[:Q] - u[H - 1 : Q - 1 : -1]
        uu = _mirror_cols(uu, desc).astype(np.float16)
        uv = _mirror_cols(uv, desc).astype(np.float16)
        vv = _mirror_cols(vv, desc)[_VROWS2].astype(np.float16)
        maps.append({"uu": uu, "uv": uv, "v": vv, "dii": dii, "div": div, "dv2": dv2})
    return maps


def _inv_perm(mode: str) -> np.ndarray:
    b = np.arange(N)
    if mode == "bfly3":
        freq = np.where(b < E, 8 * b,
               np.where(b < Q, 8 * (b - E) + 4,
               np.where(b < H, 4 * (b - Q) + 2, 2 * (b - H) + 1)))
    else:
        freq = np.where(b < Q, 4 * b,
               np.where(b < H, 4 * (b - Q) + 2, 2 * (b - H) + 1))
    inv = np.empty(N, dtype=np.int64)
    inv[freq] = b
    return inv


def kernel(x: np.ndarray) -> np.ndarray:
    x = np.asarray(x)
    assert x.shape == (B, N, N), x.shape
    nc = _get_program(MODE)
    in_maps = _make_in_maps(x, MODE)
    res = run_bass_kernel_spmd(nc, in_maps, list(range(B)))
    inv = _inv_perm(MODE)
    out = np.empty((B, N, N), dtype=np.float32)
    for i in range(B):
        zb = np.asarray(res.results[i]["z"]).astype(np.float32)
        out[i] = zb[inv][:, inv]
    return out
